# revision 73
# baseline (speedup 1.0000x reference)
"""nn_AttnA: fused QKV-proj + RMSnorm + RoPE + causal GQA attention + out-proj.

Data-parallel over the batch: core b computes batch element b (B=8 = 8 cores,
no collectives). Host pre-transposes/casts weights and x once.

Cost-model-aware v3 design (PE charges output free-size only; ACT charges
0.83ns/col + ~185ns/inst + 1283ns/table-load; DMA-XBAR transpose charges
14ns/16x128-tile on the otherwise-idle DMA device):
  1. QKV: fp16 matmuls, xT c-tiles stationary -> psum [t, 1024]
  2. RMS rstd on DVE via fast-inverse-sqrt bit trick + 2 Newton steps (no
     ACT Ln/Exp -> the only ACT func is softmax Exp -> ONE act-table load).
     q-norm multiplied into q on DVE; k-norm folded into the softmax exp's
     per-partition scale AP (score rows are key positions).
  3. RoPE on DVE in [t, o] layout; qT/kT [d, t] built by DMA-XBAR
     transposes (kT duplicated into both row halves for the odd-head
     score matmuls; no PE/psum/DVE involvement).
  4. scores per (pair, chunk, ktile): 2 matmuls -> sc psum [128k, 2, 512q];
     ONE exp per ktile over both heads [128, 2, n] with scale=rk (the 1/8
     fold makes rk exactly the Newton rsqrt output); triangle mask on
     diagonal blocks (DVE).
  5. attnV in [q, hd] layout: stationary pT [k, q-subtile], moving v_ext
     [k, 65] whose 65th column is ones -> psum [q, 65] accumulates y AND the
     softmax denominator for free. Normalize = DVE reciprocal + one fused mul.
  6. y [t,d] -> DMA-XBAR transpose -> yT [d,t]; out-proj yT t-slices
     stationary x WpT -> [t, o] fp32 -> DRAM. outproj psum rides in the sc
     tag rotation. Pipelined emission: attn(p,j) -> ytrans/outproj(prev) ->
     prep(next tau).
"""
import numpy as np
from contextlib import ExitStack

import concourse.bacc as bacc
import concourse.bass as bass
import concourse.tile as tile
from concourse import mybir
from concourse.bass_utils import run_bass_kernel_spmd
from concourse.masks import make_identity

F32 = mybir.dt.float32
F16 = mybir.dt.float16
U32 = mybir.dt.uint32
AF = mybir.ActivationFunctionType
ALU = mybir.AluOpType

DIM = 512
ROPE_BASE = 10000.0
N_CORES = 8
MAGIC = 0x5F3759DF


def build_kernel(T=2048, reps=1, variant="full"):
    P = 128
    TT = T // 128          # 16 t-tiles
    QC = T // 512          # 4 q-chunks
    NPAIR = 4

    nc = bacc.Bacc()
    xT = nc.declare_dram_parameter("xT", [DIM, T], F16, isOutput=False)
    wqkvT = nc.declare_dram_parameter("wqkvT", [DIM, 1024], F16, isOutput=False)
    wpT = nc.declare_dram_parameter("wpT", [DIM, DIM], F16, isOutput=False)
    cosd = nc.declare_dram_parameter("cosd", [T, 32], F16, isOutput=False)
    sind = nc.declare_dram_parameter("sind", [T, 32], F16, isOutput=False)
    trid = nc.declare_dram_parameter("trid", [P, P], F16, isOutput=False)
    out = nc.declare_dram_parameter("out", [T, DIM], F32, isOutput=True)

    with tile.TileContext(nc) as tc, ExitStack() as ctx:
        consts = ctx.enter_context(tc.tile_pool(name="consts", bufs=1))
        big = ctx.enter_context(tc.tile_pool(name="big", bufs=1))
        work = ctx.enter_context(tc.tile_pool(name="work", bufs=2))
        pts = ctx.enter_context(tc.tile_pool(name="pts", bufs=4))
        outp = ctx.enter_context(tc.tile_pool(name="outp", bufs=3))
        psA = ctx.enter_context(tc.tile_pool(name="psA", bufs=1, space="PSUM"))
        psS = ctx.enter_context(tc.tile_pool(name="psS", bufs=2, space="PSUM"))
        psY = ctx.enter_context(tc.tile_pool(name="psY", bufs=1, space="PSUM"))

        ident = consts.tile([P, P], F16)
        make_identity(nc, ident)
        # causal mask via PE: scores += diagNeg^T @ tri_lo adds -2000 where
        # key > query; exp then underflows those lanes to exactly 0. Keeps
        # the mask off DVE/Pool so no engine queue ever waits on an exp.
        diag_neg = consts.tile([P, P], F16)
        nc.gpsimd.tensor_scalar(diag_neg, ident, -2000.0, None, op0=ALU.mult)
        magic_t = consts.tile([P, 1], U32)
        nc.vector.memset(magic_t, MAGIC)
        tri = consts.tile([P, P], F16)
        cos_sb = consts.tile([P, TT, 32], F16)
        sin_sb = consts.tile([P, TT, 32], F16)

        xT_sb = big.tile([P, 4, T], F16)
        wqkv_sb = big.tile([P, 4, 1024], F16)
        wp_sb = big.tile([P, 4, DIM], F16)
        # per-c loads of weights + first 4 t-tiles' x columns so prep(0)'s
        # matmuls start as soon as c-tile 0 lands; cos/sin slot in after the
        # first c pair (rope needs them ~5us in), tri before x-rest (first
        # diag mask ~10us in); rest of x streams last
        for c in range(4):
            nc.sync.dma_start(out=wqkv_sb[:, c, :], in_=wqkvT[c * P:(c + 1) * P, :])
            nc.sync.dma_start(out=xT_sb[:, c, 0:512], in_=xT[c * P:(c + 1) * P, 0:512])
            if c == 0:
                nc.sync.dma_start(out=cos_sb,
                                  in_=cosd.rearrange("(tau p) i -> p tau i", p=P))
                nc.sync.dma_start(out=sin_sb,
                                  in_=sind.rearrange("(tau p) i -> p tau i", p=P))
        nc.sync.dma_start(out=tri, in_=trid[:, :])
        nc.sync.dma_start(out=xT_sb[:, :, 512:T],
                          in_=xT.rearrange("(c p) t -> p c t", p=P)[:, :, 512:T])

        # q heads are host-permuted [0,2,1,3,4,6,5,7] so slot s holds q heads
        # whose kv heads are (2*(s//2), 2*(s//2)+1) -- exactly the two row
        # halves of kT slot s//2 (no kT duplication, full-partition DMA
        # transposes only; partition-sliced transpose outs are broken on HW)
        qT_sb = big.tile([P, NPAIR, T], F16)   # slot s: rows 0:64 / 64:128
        kT_sb = big.tile([P, 2, T], F16)       # slot: kv{0,1} / kv{2,3}
        v_sb = big.tile([P, TT, 4 * 65], F16)  # per ktile: 4 kv x (64 v | 1 ones)
        yT_sb = big.tile([P, 4, T], F16)       # d-group g x t

        # ones columns of v_ext (written once; v copies skip them)
        ones_ap = bass.AP(tensor=v_sb.tensor, offset=v_sb.offset + 64,
                          ap=[v_sb.ap[0], [260, TT], [65, 4], [1, 1]])
        nc.vector.memset(ones_ap, 1.0)

        def load_wp():
            nc.sync.dma_start(out=wp_sb,
                              in_=wpT.rearrange("(c p) o -> p c o", p=P))

        def prep_ttile(tau):
            qkv_ps = psA.tile([P, 1024], F32, tag="pq", bufs=1, name="qkv_ps")
            for c in range(4):
                lhs = xT_sb[:, c, tau * P:(tau + 1) * P]
                nc.tensor.matmul(qkv_ps[:, 0:512], lhs, wqkv_sb[:, c, 0:512],
                                 start=(c == 0), stop=(c == 3))
                nc.tensor.matmul(qkv_ps[:, 512:1024], lhs, wqkv_sb[:, c, 512:1024],
                                 start=(c == 0), stop=(c == 3))
            # psum->sbuf copies: ACT while it is prep-starved (early taus),
            # DVE afterwards (DVE throughput bounds the prep pipeline)
            qk16 = work.tile([P, 768], F16, tag="qk16", bufs=4)
            vdst = bass.AP(tensor=v_sb.tensor,
                           offset=v_sb.offset + tau * 260,
                           ap=[v_sb.ap[0], [65, 4], [1, 64]])
            vsrc = qkv_ps[:, 768:1024].rearrange("p (h d) -> p h d", d=64)
            nc.scalar.activation(qk16, qkv_ps[:, 0:768], AF.Copy)
            nc.scalar.activation(vdst, vsrc, AF.Copy)
            # --- RMS stats: square on DVE for the warmup taus (fewer
            # cross-engine hops while the pipeline is latency-bound), Pool
            # afterwards (off the DVE prep path once throughput-bound) ---
            sq16 = work.tile([P, 768], F16, tag="sq16", bufs=4)
            sqeng = nc.vector if tau < 2 else nc.gpsimd
            sqeng.tensor_mul(sq16, qk16, qk16)
            ms = work.tile([P, 12], F32, tag="ms", bufs=4)
            nc.vector.tensor_reduce(ms, sq16.rearrange("p (h d) -> p h d", d=64),
                                    axis=mybir.AxisListType.X, op=ALU.add)
            # y1 = 1/sqrt(ms): bits trick seed + 1 Newton step (0.17% rel err,
            # far inside tolerance; ms ~ chi2_64 never underflows so eps-free).
            # rstd = 8*y1; with the 1/8 softmax scale the exp's k-side scale
            # is exactly y1.
            h_t = work.tile([P, 12], U32, tag="h", bufs=4)
            nc.vector.tensor_scalar(h_t, ms.bitcast(U32), 1, None,
                                    op0=ALU.logical_shift_right)
            y0b = work.tile([P, 12], U32, tag="y0b", bufs=4)
            nc.vector.tensor_tensor(y0b, magic_t.broadcast_to([P, 12]), h_t,
                                    op=ALU.subtract)
            y0 = y0b.bitcast(F32)
            t_t = work.tile([P, 12], F32, tag="t", bufs=4)
            c_t = work.tile([P, 12], F32, tag="c", bufs=4)
            y1 = work.tile([P, 12], F32, tag="y1", bufs=4)
            nc.vector.tensor_mul(t_t, ms, y0)
            nc.vector.tensor_mul(t_t, t_t, y0)
            nc.vector.tensor_scalar(c_t, t_t, -0.5, 1.5, op0=ALU.mult, op1=ALU.add)
            nc.vector.tensor_mul(y1, y0, c_t)
            # multiply rstd = 8*y1 into q AND k (in place, pre-rope; rope
            # commutes with scaling) -- scores then come out fully normed
            r16 = work.tile([P, 12], F16, tag="r16", bufs=4)
            nc.vector.tensor_scalar(r16, y1, 8.0, None, op0=ALU.mult)
            qkh = qk16.rearrange("p (h d) -> p h d", d=64)
            nc.vector.tensor_mul(qkh, qkh,
                                 r16[:, :, None].broadcast_to([P, 12, 64]))
            # --- RoPE (q normed, k raw: rotation commutes with scaling) ---
            x1, x2 = qkh[:, :, 0:32], qkh[:, :, 32:64]
            c_b = cos_sb[:, tau][:, None, :].broadcast_to([P, 12, 32])
            s_b = sin_sb[:, tau][:, None, :].broadcast_to([P, 12, 32])
            t1 = work.tile([P, 12, 32], F16, tag="t1", bufs=4)
            t2 = work.tile([P, 12, 32], F16, tag="t2", bufs=4)
            t3 = work.tile([P, 12, 32], F16, tag="t3", bufs=4)
            t4 = work.tile([P, 12, 32], F16, tag="t4", bufs=4)
            nc.vector.tensor_mul(t1, x1, c_b)
            nc.vector.tensor_mul(t2, x2, s_b)
            nc.vector.tensor_mul(t3, x1, s_b)
            nc.vector.tensor_mul(t4, x2, c_b)
            prep = work.tile([P, 768], F16, tag="prep", bufs=4)
            ph = prep.rearrange("p (h d) -> p h d", d=64)
            nc.vector.tensor_add(ph[:, :, 0:32], t1, t2)
            nc.vector.tensor_sub(ph[:, :, 32:64], t4, t3)
            # --- qT/kT via DMA-XBAR transposes (partition = d%128) ---
            nc.sync.dma_start_transpose(
                out=qT_sb[:, :, tau * P:(tau + 1) * P], in_=prep[:, 0:512])
            nc.sync.dma_start_transpose(
                out=kT_sb[:, :, tau * P:(tau + 1) * P], in_=prep[:, 512:768])

        def ycol(s, h):
            return (s // 2) * 512 + (s % 2) * 130 + h * 65

        def attn_pair_chunk(p, j, y_tile):
            nkt = 4 * j + 4
            yT_ps = psY.tile([P, 1024], F32, tag="yT", bufs=1, name="yT_ps")
            kvs = p // 2          # kT slot; kv heads (2*kvs, 2*kvs+1)
            for k in range(nkt):
                offs = max(0, P * (k - 4 * j))
                n = 512 - offs
                kcol = k * P
                qcol = 512 * j + offs
                diag = k >= 4 * j
                sc = psS.tile([P, 1024], F32, tag="sc", bufs=2, name="sc")
                nc.tensor.matmul(sc[:, offs:512],
                                 kT_sb[0:64, kvs, kcol:kcol + P],
                                 qT_sb[0:64, p, qcol:qcol + n],
                                 start=True, stop=not diag)
                nc.tensor.matmul(sc[:, 512 + offs:1024],
                                 kT_sb[64:128, kvs, kcol:kcol + P],
                                 qT_sb[64:128, p, qcol:qcol + n],
                                 start=True, stop=not diag, tile_position=(64, 0))
                if diag:  # add -2000 above the diagonal of the diag square
                    nc.tensor.matmul(sc[:, offs:offs + P], diag_neg, tri,
                                     start=False, stop=True)
                    nc.tensor.matmul(sc[:, 512 + offs:512 + offs + P],
                                     diag_neg, tri, start=False, stop=True)
                pT = pts.tile([P, 1024], F16, tag="pT", bufs=4, name="pT")
                sc_v = bass.AP(tensor=sc.tensor, offset=sc.offset + offs,
                               ap=[sc.ap[0], [512, 2], [1, n]])
                pT_v = bass.AP(tensor=pT.tensor, offset=pT.offset + offs,
                               ap=[pT.ap[0], [512, 2], [1, n]])
                nc.scalar.activation(pT_v, sc_v, AF.Exp, scale=0.125)
                s0 = max(0, k - 4 * j)
                for s in range(s0, 4):
                    for h in range(2):
                        # one start/stop per psum BANK (banks hold subtile
                        # pairs {0,1} and {2,3}); other chains accumulate
                        # into the bank's pending-zero bytes.
                        st = (k == 0) and h == 0 and (s % 2 == 0)
                        sp = (k == 4 * j + s) and h == 1 and (s % 2 == 1)
                        kv = 2 * kvs + h
                        nc.tensor.matmul(
                            yT_ps[:, ycol(s, h):ycol(s, h) + 65],
                            pT[:, h * 512 + s * P: h * 512 + (s + 1) * P],
                            v_sb[:, k, kv * 65:(kv + 1) * 65],
                            start=st, stop=sp)
            # --- normalize: y / den, den at column 64 of each (s,h) block ---
            rd = work.tile([P, 8], F32, tag="rd")
            den_ap = bass.AP(tensor=yT_ps.tensor, offset=yT_ps.offset + 64,
                             ap=[yT_ps.ap[0], [512, 2], [130, 2], [65, 2], [1, 1]])
            nc.vector.reciprocal(rd, den_ap)
            yv_ap = bass.AP(tensor=yT_ps.tensor, offset=yT_ps.offset,
                            ap=[yT_ps.ap[0], [512, 2], [130, 2], [65, 2], [1, 64]])
            rd_b = bass.AP(tensor=rd.tensor, offset=rd.offset,
                           ap=[rd.ap[0], [4, 2], [2, 2], [1, 2], [0, 64]])
            ydst = bass.AP(tensor=y_tile.tensor,
                           offset=y_tile.offset + p * P,
                           ap=[y_tile.ap[0], [1024, 2], [512, 2], [64, 2], [1, 64]])
            nc.vector.tensor_mul(ydst, yv_ap, rd_b)

        def ytrans_outproj(u, y_tile):
            s = u % 4
            nc.sync.dma_start_transpose(
                out=yT_sb[:, :, u * P:(u + 1) * P], in_=y_tile[:, s, :])
            op_ps = psS.tile([P, 1024], F32, tag="sc", bufs=2, name="op_ps")
            for g in range(4):
                nc.tensor.matmul(op_ps[:, 0:512],
                                 yT_sb[:, g, u * P:(u + 1) * P],
                                 wp_sb[:, g, :], start=(g == 0), stop=(g == 3))
            o32 = outp.tile([P, 512], F32, tag="o32")
            nc.vector.tensor_copy(o32, op_ps[:, 0:512])
            nc.sync.dma_start(out=out[u * P:(u + 1) * P, :], in_=o32)

        def tail_outproj(y_tile):
            # last chunk: phased for maximum overlap. All 16 PE transposes
            # land in ONE full-slab psum tile (no rotation stalls); copies
            # split across ACT+DVE; the two sc slots run two op chains.
            yTT = psA.tile([P, 2048], F16, tag="pq", bufs=1, name="yTT")
            for s in range(4):
                for g in range(4):
                    nc.tensor.transpose(
                        yTT[:, s * 512 + g * P: s * 512 + (g + 1) * P],
                        y_tile[:, s, g * P:(g + 1) * P], ident)
            for s in range(4):
                u = 4 * (QC - 1) + s
                ytdst = bass.AP(tensor=yT_sb.tensor,
                                offset=yT_sb.offset + u * P,
                                ap=[yT_sb.ap[0], [T, 4], [1, P]])
                src = yTT[:, s * 512:(s + 1) * 512].rearrange(
                    "p (g t) -> p g t", t=P)
                nc.scalar.activation(ytdst, src, AF.Copy)
            op_tiles = []
            for s in range(4):
                u = 4 * (QC - 1) + s
                op_ps = psS.tile([P, 1024], F32, tag="sc", bufs=2,
                                 name=f"op_t{s}")
                op_tiles.append(op_ps)
                for g in range(4):
                    nc.tensor.matmul(op_ps[:, 0:512],
                                     yT_sb[:, g, u * P:(u + 1) * P],
                                     wp_sb[:, g, :], start=(g == 0),
                                     stop=(g == 3))
                o32 = outp.tile([P, 512], F32, tag="o32")
                nc.vector.tensor_copy(o32, op_ps[:, 0:512])
                nc.sync.dma_start(out=out[u * P:(u + 1) * P, :], in_=o32)

        for _rep in range(reps):
            y_tiles = {}
            for tau in range(4):
                prep_ttile(tau)
            load_wp()
            # next-chunk preps run 2-wide on the chunk's first step so the
            # last tau lands well before the next chunk's first scores
            prep_sched = {(j, 0): [4 * j + 4, 4 * j + 5] for j in range(3)}
            prep_sched.update({(j, 1): [4 * j + 6] for j in range(3)})
            prep_sched.update({(j, 2): [4 * j + 7] for j in range(3)})
            for j in range(QC):
                y_tiles[j] = work.tile([P, 4, 512], F16, tag="ych", bufs=3,
                                       name=f"y_ch{j}")
                for p in range(NPAIR):
                    attn_pair_chunk(p, j, y_tiles[j])
                    if j > 0:
                        ytrans_outproj(4 * (j - 1) + p, y_tiles[j - 1])
                    for nxt in prep_sched.get((j, p), []):
                        prep_ttile(nxt)
            tail_outproj(y_tiles[QC - 1])

    nc.finalize()
    return nc


_NC_CACHE = {}


def _get_nc(T=2048, reps=1):
    key = (T, reps)
    if key not in _NC_CACHE:
        _NC_CACHE[key] = build_kernel(T=T, reps=reps)
    return _NC_CACHE[key]


def make_host_inputs(x_b, wqkvT, wpT, cosd, sind, trid):
    return dict(xT=np.ascontiguousarray(x_b.T).astype(np.float16),
                wqkvT=wqkvT, wpT=wpT, cosd=cosd, sind=sind, trid=trid)


HEAD_PERM = [0, 2, 1, 3, 4, 6, 5, 7]  # slot s: heads (perm[2s], perm[2s+1])


def make_shared_inputs(Wq, Wk, Wv, Wp, T):
    # permute q heads so each qT slot's two heads use the two kv heads that
    # sit in the two row halves of a kT slot; Wp's input rows get the same
    # permutation so y can stay in permuted order end-to-end
    pidx = np.concatenate([np.arange(h * 64, (h + 1) * 64) for h in HEAD_PERM])
    wqkvT = np.ascontiguousarray(
        np.concatenate([Wq[pidx], Wk, Wv], 0).T).astype(np.float16)
    wpT = np.ascontiguousarray(Wp.T[pidx]).astype(np.float16)
    inv = 1.0 / (ROPE_BASE ** (np.arange(0, 64, 2) / 64))
    f = np.outer(np.arange(T), inv)
    cosd = np.cos(f).astype(np.float16)
    sind = np.sin(f).astype(np.float16)
    # strict lower triangle [key > query]: the masked-out region
    trid = (np.arange(128)[None, :] < np.arange(128)[:, None]).astype(np.float16)
    return wqkvT, wpT, cosd, sind, trid


def kernel(x, Wq, Wk, Wv, Wp, reps=1):
    x = np.asarray(x)
    B, T, C = x.shape
    assert (B, C) == (N_CORES, DIM)
    nc = _get_nc(T=T, reps=reps)
    shared = make_shared_inputs(np.asarray(Wq), np.asarray(Wk),
                                np.asarray(Wv), np.asarray(Wp), T)
    in_maps = [make_host_inputs(x[b], *shared) for b in range(B)]
    res = run_bass_kernel_spmd(nc, in_maps, list(range(N_CORES)))
    return np.stack([res.results[b]["out"] for b in range(B)]).astype(np.float32)


# revision 76
# speedup vs baseline: 1.0018x; 1.0018x over previous
"""nn_AttnA: fused QKV-proj + RMSnorm + RoPE + causal GQA attention + out-proj.

Data-parallel over the batch: core b computes batch element b (B=8 = 8 cores,
no collectives). Host pre-transposes/casts weights and x once.

Cost-model-aware v3 design (PE charges output free-size only; ACT charges
0.83ns/col + ~185ns/inst + 1283ns/table-load; DMA-XBAR transpose charges
14ns/16x128-tile on the otherwise-idle DMA device):
  1. QKV: fp16 matmuls, xT c-tiles stationary -> psum [t, 1024]
  2. RMS rstd on DVE via fast-inverse-sqrt bit trick + 2 Newton steps (no
     ACT Ln/Exp -> the only ACT func is softmax Exp -> ONE act-table load).
     q-norm multiplied into q on DVE; k-norm folded into the softmax exp's
     per-partition scale AP (score rows are key positions).
  3. RoPE on DVE in [t, o] layout; qT/kT [d, t] built by DMA-XBAR
     transposes (kT duplicated into both row halves for the odd-head
     score matmuls; no PE/psum/DVE involvement).
  4. scores per (pair, chunk, ktile): 2 matmuls -> sc psum [128k, 2, 512q];
     ONE exp per ktile over both heads [128, 2, n] with scale=rk (the 1/8
     fold makes rk exactly the Newton rsqrt output); triangle mask on
     diagonal blocks (DVE).
  5. attnV in [q, hd] layout: stationary pT [k, q-subtile], moving v_ext
     [k, 65] whose 65th column is ones -> psum [q, 65] accumulates y AND the
     softmax denominator for free. Normalize = DVE reciprocal + one fused mul.
  6. y [t,d] -> DMA-XBAR transpose -> yT [d,t]; out-proj yT t-slices
     stationary x WpT -> [t, o] fp32 -> DRAM. outproj psum rides in the sc
     tag rotation. Pipelined emission: attn(p,j) -> ytrans/outproj(prev) ->
     prep(next tau).
"""
import numpy as np
from contextlib import ExitStack

import concourse.bacc as bacc
import concourse.bass as bass
import concourse.tile as tile
from concourse import mybir
from concourse.bass_utils import run_bass_kernel_spmd
from concourse.masks import make_identity

F32 = mybir.dt.float32
F16 = mybir.dt.float16
U32 = mybir.dt.uint32
AF = mybir.ActivationFunctionType
ALU = mybir.AluOpType

DIM = 512
ROPE_BASE = 10000.0
N_CORES = 8
MAGIC = 0x5F3759DF


def build_kernel(T=2048, reps=1, variant="full"):
    P = 128
    TT = T // 128          # 16 t-tiles
    QC = T // 512          # 4 q-chunks
    NPAIR = 4

    nc = bacc.Bacc()
    xT = nc.declare_dram_parameter("xT", [DIM, T], F16, isOutput=False)
    wqkvT = nc.declare_dram_parameter("wqkvT", [DIM, 1024], F16, isOutput=False)
    wpT = nc.declare_dram_parameter("wpT", [DIM, DIM], F16, isOutput=False)
    cosd = nc.declare_dram_parameter("cosd", [T, 32], F16, isOutput=False)
    sind = nc.declare_dram_parameter("sind", [T, 32], F16, isOutput=False)
    trid = nc.declare_dram_parameter("trid", [P, P], F16, isOutput=False)
    out = nc.declare_dram_parameter("out", [T, DIM], F32, isOutput=True)

    with tile.TileContext(nc) as tc, ExitStack() as ctx:
        consts = ctx.enter_context(tc.tile_pool(name="consts", bufs=1))
        big = ctx.enter_context(tc.tile_pool(name="big", bufs=1))
        work = ctx.enter_context(tc.tile_pool(name="work", bufs=2))
        pts = ctx.enter_context(tc.tile_pool(name="pts", bufs=6))
        outp = ctx.enter_context(tc.tile_pool(name="outp", bufs=3))
        psA = ctx.enter_context(tc.tile_pool(name="psA", bufs=1, space="PSUM"))
        psS = ctx.enter_context(tc.tile_pool(name="psS", bufs=2, space="PSUM"))
        psY = ctx.enter_context(tc.tile_pool(name="psY", bufs=1, space="PSUM"))

        ident = consts.tile([P, P], F16)
        make_identity(nc, ident)
        # causal mask via PE: scores += diagNeg^T @ tri_lo adds -2000 where
        # key > query; exp then underflows those lanes to exactly 0. Keeps
        # the mask off DVE/Pool so no engine queue ever waits on an exp.
        diag_neg = consts.tile([P, P], F16)
        nc.gpsimd.tensor_scalar(diag_neg, ident, -2000.0, None, op0=ALU.mult)
        magic_t = consts.tile([P, 1], U32)
        nc.vector.memset(magic_t, MAGIC)
        tri = consts.tile([P, P], F16)
        cos_sb = consts.tile([P, TT, 32], F16)
        sin_sb = consts.tile([P, TT, 32], F16)

        xT_sb = big.tile([P, 4, T], F16)
        wqkv_sb = big.tile([P, 4, 1024], F16)
        wp_sb = big.tile([P, 4, DIM], F16)
        # per-c loads of weights + first 4 t-tiles' x columns so prep(0)'s
        # matmuls start as soon as c-tile 0 lands; cos/sin slot in after the
        # first c pair (rope needs them ~5us in), tri before x-rest (first
        # diag mask ~10us in); rest of x streams last
        for c in range(4):
            nc.sync.dma_start(out=wqkv_sb[:, c, :], in_=wqkvT[c * P:(c + 1) * P, :])
            nc.sync.dma_start(out=xT_sb[:, c, 0:512], in_=xT[c * P:(c + 1) * P, 0:512])
            if c == 0:
                nc.sync.dma_start(out=cos_sb,
                                  in_=cosd.rearrange("(tau p) i -> p tau i", p=P))
                nc.sync.dma_start(out=sin_sb,
                                  in_=sind.rearrange("(tau p) i -> p tau i", p=P))
        nc.sync.dma_start(out=tri, in_=trid[:, :])
        nc.sync.dma_start(out=xT_sb[:, :, 512:T],
                          in_=xT.rearrange("(c p) t -> p c t", p=P)[:, :, 512:T])

        # q heads are host-permuted [0,2,1,3,4,6,5,7] so slot s holds q heads
        # whose kv heads are (2*(s//2), 2*(s//2)+1) -- exactly the two row
        # halves of kT slot s//2 (no kT duplication, full-partition DMA
        # transposes only; partition-sliced transpose outs are broken on HW)
        qT_sb = big.tile([P, NPAIR, T], F16)   # slot s: rows 0:64 / 64:128
        kT_sb = big.tile([P, 2, T], F16)       # slot: kv{0,1} / kv{2,3}
        v_sb = big.tile([P, TT, 4 * 65], F16)  # per ktile: 4 kv x (64 v | 1 ones)
        yT_sb = big.tile([P, 4, T], F16)       # d-group g x t

        # ones columns of v_ext (written once; v copies skip them)
        ones_ap = bass.AP(tensor=v_sb.tensor, offset=v_sb.offset + 64,
                          ap=[v_sb.ap[0], [260, TT], [65, 4], [1, 1]])
        nc.vector.memset(ones_ap, 1.0)

        def load_wp():
            nc.sync.dma_start(out=wp_sb,
                              in_=wpT.rearrange("(c p) o -> p c o", p=P))

        def prep_ttile(tau):
            qkv_ps = psA.tile([P, 1024], F32, tag="pq", bufs=1, name="qkv_ps")
            for c in range(4):
                lhs = xT_sb[:, c, tau * P:(tau + 1) * P]
                nc.tensor.matmul(qkv_ps[:, 0:512], lhs, wqkv_sb[:, c, 0:512],
                                 start=(c == 0), stop=(c == 3))
                nc.tensor.matmul(qkv_ps[:, 512:1024], lhs, wqkv_sb[:, c, 512:1024],
                                 start=(c == 0), stop=(c == 3))
            # psum->sbuf copies: ACT while it is prep-starved (early taus),
            # DVE afterwards (DVE throughput bounds the prep pipeline)
            qk16 = work.tile([P, 768], F16, tag="qk16", bufs=6)
            vdst = bass.AP(tensor=v_sb.tensor,
                           offset=v_sb.offset + tau * 260,
                           ap=[v_sb.ap[0], [65, 4], [1, 64]])
            vsrc = qkv_ps[:, 768:1024].rearrange("p (h d) -> p h d", d=64)
            nc.scalar.activation(qk16, qkv_ps[:, 0:768], AF.Copy)
            nc.scalar.activation(vdst, vsrc, AF.Copy)
            # --- RMS stats: square on DVE for the warmup taus (fewer
            # cross-engine hops while the pipeline is latency-bound), Pool
            # afterwards (off the DVE prep path once throughput-bound) ---
            sq16 = work.tile([P, 768], F16, tag="sq16", bufs=6)
            sqeng = nc.vector if tau < 2 else nc.gpsimd
            sqeng.tensor_mul(sq16, qk16, qk16)
            ms = work.tile([P, 12], F32, tag="ms", bufs=6)
            nc.vector.tensor_reduce(ms, sq16.rearrange("p (h d) -> p h d", d=64),
                                    axis=mybir.AxisListType.X, op=ALU.add)
            # y1 = 1/sqrt(ms): bits trick seed + 1 Newton step (0.17% rel err,
            # far inside tolerance; ms ~ chi2_64 never underflows so eps-free).
            # rstd = 8*y1; with the 1/8 softmax scale the exp's k-side scale
            # is exactly y1.
            h_t = work.tile([P, 12], U32, tag="h", bufs=6)
            nc.vector.tensor_scalar(h_t, ms.bitcast(U32), 1, None,
                                    op0=ALU.logical_shift_right)
            y0b = work.tile([P, 12], U32, tag="y0b", bufs=6)
            nc.vector.tensor_tensor(y0b, magic_t.broadcast_to([P, 12]), h_t,
                                    op=ALU.subtract)
            y0 = y0b.bitcast(F32)
            t_t = work.tile([P, 12], F32, tag="t", bufs=6)
            c_t = work.tile([P, 12], F32, tag="c", bufs=6)
            y1 = work.tile([P, 12], F32, tag="y1", bufs=6)
            nc.vector.tensor_mul(t_t, ms, y0)
            nc.vector.tensor_mul(t_t, t_t, y0)
            nc.vector.tensor_scalar(c_t, t_t, -0.5, 1.5, op0=ALU.mult, op1=ALU.add)
            nc.vector.tensor_mul(y1, y0, c_t)
            # multiply rstd = 8*y1 into q AND k (in place, pre-rope; rope
            # commutes with scaling) -- scores then come out fully normed
            r16 = work.tile([P, 12], F16, tag="r16", bufs=6)
            nc.vector.tensor_scalar(r16, y1, 8.0, None, op0=ALU.mult)
            qkh = qk16.rearrange("p (h d) -> p h d", d=64)
            nc.vector.tensor_mul(qkh, qkh,
                                 r16[:, :, None].broadcast_to([P, 12, 64]))
            # --- RoPE (q normed, k raw: rotation commutes with scaling) ---
            x1, x2 = qkh[:, :, 0:32], qkh[:, :, 32:64]
            c_b = cos_sb[:, tau][:, None, :].broadcast_to([P, 12, 32])
            s_b = sin_sb[:, tau][:, None, :].broadcast_to([P, 12, 32])
            t1 = work.tile([P, 12, 32], F16, tag="t1", bufs=6)
            t2 = work.tile([P, 12, 32], F16, tag="t2", bufs=6)
            t3 = work.tile([P, 12, 32], F16, tag="t3", bufs=6)
            t4 = work.tile([P, 12, 32], F16, tag="t4", bufs=6)
            nc.vector.tensor_mul(t1, x1, c_b)
            nc.vector.tensor_mul(t2, x2, s_b)
            nc.vector.tensor_mul(t3, x1, s_b)
            nc.vector.tensor_mul(t4, x2, c_b)
            prep = work.tile([P, 768], F16, tag="prep", bufs=6)
            ph = prep.rearrange("p (h d) -> p h d", d=64)
            nc.vector.tensor_add(ph[:, :, 0:32], t1, t2)
            nc.vector.tensor_sub(ph[:, :, 32:64], t4, t3)
            # --- qT/kT via DMA-XBAR transposes (partition = d%128) ---
            nc.sync.dma_start_transpose(
                out=qT_sb[:, :, tau * P:(tau + 1) * P], in_=prep[:, 0:512])
            nc.sync.dma_start_transpose(
                out=kT_sb[:, :, tau * P:(tau + 1) * P], in_=prep[:, 512:768])

        def ycol(s, h):
            return (s // 2) * 512 + (s % 2) * 130 + h * 65

        def attn_pair_chunk(p, j, y_tile):
            nkt = 4 * j + 4
            yT_ps = psY.tile([P, 1024], F32, tag="yT", bufs=1, name="yT_ps")
            kvs = p // 2          # kT slot; kv heads (2*kvs, 2*kvs+1)
            for k in range(nkt):
                offs = max(0, P * (k - 4 * j))
                n = 512 - offs
                kcol = k * P
                qcol = 512 * j + offs
                diag = k >= 4 * j
                sc = psS.tile([P, 1024], F32, tag="sc", bufs=2, name="sc")
                nc.tensor.matmul(sc[:, offs:512],
                                 kT_sb[0:64, kvs, kcol:kcol + P],
                                 qT_sb[0:64, p, qcol:qcol + n],
                                 start=True, stop=not diag)
                nc.tensor.matmul(sc[:, 512 + offs:1024],
                                 kT_sb[64:128, kvs, kcol:kcol + P],
                                 qT_sb[64:128, p, qcol:qcol + n],
                                 start=True, stop=not diag, tile_position=(64, 0))
                if diag:  # add -2000 above the diagonal of the diag square
                    nc.tensor.matmul(sc[:, offs:offs + P], diag_neg, tri,
                                     start=False, stop=True)
                    nc.tensor.matmul(sc[:, 512 + offs:512 + offs + P],
                                     diag_neg, tri, start=False, stop=True)
                pT = pts.tile([P, 1024], F16, tag="pT", bufs=6, name="pT")
                sc_v = bass.AP(tensor=sc.tensor, offset=sc.offset + offs,
                               ap=[sc.ap[0], [512, 2], [1, n]])
                pT_v = bass.AP(tensor=pT.tensor, offset=pT.offset + offs,
                               ap=[pT.ap[0], [512, 2], [1, n]])
                nc.scalar.activation(pT_v, sc_v, AF.Exp, scale=0.125)
                s0 = max(0, k - 4 * j)
                for s in range(s0, 4):
                    for h in range(2):
                        # one start/stop per psum BANK (banks hold subtile
                        # pairs {0,1} and {2,3}); other chains accumulate
                        # into the bank's pending-zero bytes.
                        st = (k == 0) and h == 0 and (s % 2 == 0)
                        sp = (k == 4 * j + s) and h == 1 and (s % 2 == 1)
                        kv = 2 * kvs + h
                        nc.tensor.matmul(
                            yT_ps[:, ycol(s, h):ycol(s, h) + 65],
                            pT[:, h * 512 + s * P: h * 512 + (s + 1) * P],
                            v_sb[:, k, kv * 65:(kv + 1) * 65],
                            start=st, stop=sp)
            # --- normalize: y / den, den at column 64 of each (s,h) block ---
            rd = work.tile([P, 8], F32, tag="rd")
            den_ap = bass.AP(tensor=yT_ps.tensor, offset=yT_ps.offset + 64,
                             ap=[yT_ps.ap[0], [512, 2], [130, 2], [65, 2], [1, 1]])
            nc.vector.reciprocal(rd, den_ap)
            yv_ap = bass.AP(tensor=yT_ps.tensor, offset=yT_ps.offset,
                            ap=[yT_ps.ap[0], [512, 2], [130, 2], [65, 2], [1, 64]])
            rd_b = bass.AP(tensor=rd.tensor, offset=rd.offset,
                           ap=[rd.ap[0], [4, 2], [2, 2], [1, 2], [0, 64]])
            ydst = bass.AP(tensor=y_tile.tensor,
                           offset=y_tile.offset + p * P,
                           ap=[y_tile.ap[0], [1024, 2], [512, 2], [64, 2], [1, 64]])
            nc.vector.tensor_mul(ydst, yv_ap, rd_b)

        def ytrans_outproj(u, y_tile):
            s = u % 4
            nc.sync.dma_start_transpose(
                out=yT_sb[:, :, u * P:(u + 1) * P], in_=y_tile[:, s, :])
            op_ps = psS.tile([P, 1024], F32, tag="sc", bufs=2, name="op_ps")
            for g in range(4):
                nc.tensor.matmul(op_ps[:, 0:512],
                                 yT_sb[:, g, u * P:(u + 1) * P],
                                 wp_sb[:, g, :], start=(g == 0), stop=(g == 3))
            o32 = outp.tile([P, 512], F32, tag="o32")
            nc.vector.tensor_copy(o32, op_ps[:, 0:512])
            nc.sync.dma_start(out=out[u * P:(u + 1) * P, :], in_=o32)

        def tail_outproj(y_tile):
            # last chunk: phased for maximum overlap. All 16 PE transposes
            # land in ONE full-slab psum tile (no rotation stalls); copies
            # split across ACT+DVE; the two sc slots run two op chains.
            yTT = psA.tile([P, 2048], F16, tag="pq", bufs=1, name="yTT")
            for s in range(4):
                for g in range(4):
                    nc.tensor.transpose(
                        yTT[:, s * 512 + g * P: s * 512 + (g + 1) * P],
                        y_tile[:, s, g * P:(g + 1) * P], ident)
            for s in range(4):
                u = 4 * (QC - 1) + s
                ytdst = bass.AP(tensor=yT_sb.tensor,
                                offset=yT_sb.offset + u * P,
                                ap=[yT_sb.ap[0], [T, 4], [1, P]])
                src = yTT[:, s * 512:(s + 1) * 512].rearrange(
                    "p (g t) -> p g t", t=P)
                nc.scalar.activation(ytdst, src, AF.Copy)
            op_tiles = []
            for s in range(4):
                u = 4 * (QC - 1) + s
                op_ps = psS.tile([P, 1024], F32, tag="sc", bufs=2,
                                 name=f"op_t{s}")
                op_tiles.append(op_ps)
                for g in range(4):
                    nc.tensor.matmul(op_ps[:, 0:512],
                                     yT_sb[:, g, u * P:(u + 1) * P],
                                     wp_sb[:, g, :], start=(g == 0),
                                     stop=(g == 3))
                o32 = outp.tile([P, 512], F32, tag="o32")
                nc.vector.tensor_copy(o32, op_ps[:, 0:512])
                nc.sync.dma_start(out=out[u * P:(u + 1) * P, :], in_=o32)

        for _rep in range(reps):
            y_tiles = {}
            for tau in range(4):
                prep_ttile(tau)
            load_wp()
            # next-chunk preps run 2-wide on the chunk's first step so the
            # last tau lands well before the next chunk's first scores
            prep_sched = {(j, 0): [4 * j + 4, 4 * j + 5] for j in range(3)}
            prep_sched.update({(j, 1): [4 * j + 6] for j in range(3)})
            prep_sched.update({(j, 2): [4 * j + 7] for j in range(3)})
            for j in range(QC):
                y_tiles[j] = work.tile([P, 4, 512], F16, tag="ych", bufs=3,
                                       name=f"y_ch{j}")
                for p in range(NPAIR):
                    attn_pair_chunk(p, j, y_tiles[j])
                    if j > 0:
                        ytrans_outproj(4 * (j - 1) + p, y_tiles[j - 1])
                    for nxt in prep_sched.get((j, p), []):
                        prep_ttile(nxt)
            tail_outproj(y_tiles[QC - 1])

    nc.finalize()
    return nc


_NC_CACHE = {}


def _get_nc(T=2048, reps=1):
    key = (T, reps)
    if key not in _NC_CACHE:
        _NC_CACHE[key] = build_kernel(T=T, reps=reps)
    return _NC_CACHE[key]


def make_host_inputs(x_b, wqkvT, wpT, cosd, sind, trid):
    return dict(xT=np.ascontiguousarray(x_b.T).astype(np.float16),
                wqkvT=wqkvT, wpT=wpT, cosd=cosd, sind=sind, trid=trid)


HEAD_PERM = [0, 2, 1, 3, 4, 6, 5, 7]  # slot s: heads (perm[2s], perm[2s+1])


def make_shared_inputs(Wq, Wk, Wv, Wp, T):
    # permute q heads so each qT slot's two heads use the two kv heads that
    # sit in the two row halves of a kT slot; Wp's input rows get the same
    # permutation so y can stay in permuted order end-to-end
    pidx = np.concatenate([np.arange(h * 64, (h + 1) * 64) for h in HEAD_PERM])
    wqkvT = np.ascontiguousarray(
        np.concatenate([Wq[pidx], Wk, Wv], 0).T).astype(np.float16)
    wpT = np.ascontiguousarray(Wp.T[pidx]).astype(np.float16)
    inv = 1.0 / (ROPE_BASE ** (np.arange(0, 64, 2) / 64))
    f = np.outer(np.arange(T), inv)
    cosd = np.cos(f).astype(np.float16)
    sind = np.sin(f).astype(np.float16)
    # strict lower triangle [key > query]: the masked-out region
    trid = (np.arange(128)[None, :] < np.arange(128)[:, None]).astype(np.float16)
    return wqkvT, wpT, cosd, sind, trid


def kernel(x, Wq, Wk, Wv, Wp, reps=1):
    x = np.asarray(x)
    B, T, C = x.shape
    assert (B, C) == (N_CORES, DIM)
    nc = _get_nc(T=T, reps=reps)
    shared = make_shared_inputs(np.asarray(Wq), np.asarray(Wk),
                                np.asarray(Wv), np.asarray(Wp), T)
    in_maps = [make_host_inputs(x[b], *shared) for b in range(B)]
    res = run_bass_kernel_spmd(nc, in_maps, list(range(N_CORES)))
    return np.stack([res.results[b]["out"] for b in range(B)]).astype(np.float32)
